# revision 5
# baseline (speedup 1.0000x reference)
"""Trainium2 Bass kernel for nn_AttentionLayer: softmax(Q K^T / sqrt(d)).

Data-parallel over batch: 8 batch elements -> 8 NeuronCores, weights
replicated, no collectives.

Algebraic restructure (exact, softmax-invariant): with q = x Wq + bq and
k = x Wk + bk,
    q k^T = x (Wq Wk^T) x^T  +  1 (x Wk bq)^T  +  [terms constant along n]
and row-softmax drops any per-row constant, so
    alpha = softmax_n( (t x^T) / sqrt(d) ),   t = x W' + 1 c2^T,
    W' = Wq Wk^T  (512x512),  c2 = Wk bq.
This replaces one of the two [2048x512x512] projections with a single
[512x512x512] matmul (W') — ~6.6us less PE work per core — and removes
the bk load entirely.

Per core:
  xT    = transpose(x)            (PE f32 transposes, DVE evict->bf16)
  WqT/WkT = transpose(Wq/Wk)      (PE f32 transposes, ACT evict->bf16)
  W'    = WqT^T @ WkT             (TensorE bf16, ACT evict->bf16)
  c2    = WkT^T @ bq              (16 tiny N=1 matmuls, f32 PSUM accum)
  tT    = W'-chunks @ xT + c2     (TensorE bf16, bias evict ACT/DVE)
  S     = tT^T @ xT               (TensorE bf16, accumulate over f-tiles)
  E     = exp(S / sqrt(d)) with fused row-sum accumulate (ACT)
  out   = E / rowsum              (DVE per-partition scalar mul -> bf16)

Schedule notes (from NTFF traces): PE matmul throughput is at roofline
when dense (216ns per 512-wide bf16 MM); the two measured loss sources
beyond PE work are (a) the input stream — a single issuing engine's DMA
queue delivers only ~100GB/s and a single transfer ~25GB/s, so the 6MB
input is split into 128KB sub-transfers spread over the SP, ACT and
GpSimd(SWDGE) queues with per-queue phase chaining so concurrency stays
high and x group 0 lands early — and (b) the end-of-kernel semaphore
teardown (~35-115ns per allocated semaphore, serialized), so instruction
and DMA counts are kept low: all PSUM lives in one [P,2,512] 2-bank tag
(8 banks, bufs=4), transpose/W'/wT evictions move [P,2,512] at a time,
and each m-tile issues ONE merged [128,2048] output DMA (SP and GpSimd
alternate; ACT stays exp-only in the scores phase so the epilogue keeps
pace with the PE). The DRAM output is bf16 (halves the ~17MB/core output
stream) and is upconverted to f32 on the host; rel err vs the fp32
reference is ~4.5e-3.
"""

import os
import sys

sys.path.insert(0, "/opt/trn_rl_repo")

import numpy as np

import concourse.mybir as mybir
import concourse.tile as tile
from concourse import bacc
from concourse.bass_utils import run_bass_kernel_spmd
from concourse.masks import make_identity

B, S, F, D = 8, 2048, 512, 512
P = 128
ST = S // P   # 16 s-tiles
FT = F // P   # 4  f-tiles (contraction for projections / scores)
NCH = 512     # moving-operand / PSUM-bank chunk along the free axis
SC = S // NCH  # 4 chunks of the s axis

F32 = mybir.dt.float32
BF16 = mybir.dt.bfloat16

WARMUP_MMS = int(os.environ.get("BASS_ATTN_WARMUP", "6"))
OUT_BF16 = os.environ.get("BASS_ATTN_OUT_BF16", "1") == "1"


def _emit(nc, tc, ctx, x_ext, wq_ext, wk_ext, bq_ext, out_ext):
    Act = mybir.ActivationFunctionType

    consts = ctx.enter_context(tc.tile_pool(name="consts", bufs=1))
    persist = ctx.enter_context(tc.tile_pool(name="persist", bufs=1))
    xstage = ctx.enter_context(tc.tile_pool(name="xstage", bufs=4))
    psum = ctx.enter_context(tc.tile_pool(name="psum", bufs=4, space="PSUM"))
    epool = ctx.enter_context(tc.tile_pool(name="epool", bufs=2))
    opool = ctx.enter_context(tc.tile_pool(name="opool", bufs=2))
    spool = ctx.enter_context(tc.tile_pool(name="spool", bufs=4))

    def ps_tile(name):
        # single unified PSUM tag: 4 bufs x [P, 2, 512] f32 = all 8 banks
        return psum.tile([P, 2, NCH], F32, tag="ps", bufs=4, name=name)

    ident = consts.tile([P, P], F32)
    make_identity(nc, ident[:])
    # --- PE warmup: garbage matmuls while input DMAs land (HAM -> K=8/8)
    if WARMUP_MMS:
        wrm = consts.tile([P, 256], F32)
        nc.gpsimd.memset(wrm[:], 0.0)
        wps = ps_tile("warmps")
        for _ in range(WARMUP_MMS):
            nc.tensor.matmul(wps[:, 0, :256], ident[:], wrm[:], start=True, stop=True)

    from concourse.tile import add_dep_helper

    def gate(first_insts, prev_insts):
        for fi in first_insts:
            for pi in prev_insts:
                add_dep_helper(fi.ins, pi.ins, reason="input DMA phase chain")

    # --- input streaming: per-queue concurrency is the binding constraint
    # (a single transfer moves ~25GB/s, one issuing engine's queue ~100GB/s),
    # so split each [128,512] x tile into two 64-row transfers and spread them:
    # rows 0-63 of each tile on SP, rows 64-127 on GpSimd, weights on ACT.
    def load_x_group_half(t, sg, eng, half):
        insts = []
        lo, hi = (0, 64) if half == 0 else (64, P)
        for j in range(4):
            st = sg * 4 + j
            insts.append(
                eng.dma_start(
                    t[lo:hi, j, :], x_ext.ap()[st * P + lo : st * P + hi, :]
                )
            )
        return insts

    xgroups = {}
    sp_prev = None
    gp_prev = None
    for sg in range(SC):
        xgroups[sg] = xstage.tile([P, 4, F], F32, tag="xstage", bufs=4, name=f"xg{sg}")
        sp_insts = load_x_group_half(xgroups[sg], sg, nc.sync, 0)
        gp_insts = load_x_group_half(xgroups[sg], sg, nc.gpsimd, 1)
        if sp_prev is not None:
            gate(sp_insts[:1], sp_prev)
            gate(gp_insts[:1], gp_prev)
        sp_prev, gp_prev = sp_insts, gp_insts
        if sg == 0:
            # Wk on the SP chain right after xg0 (4 sub-transfers)
            wk_st = xstage.tile([P, FT, D], F32, tag="wstage", bufs=2, name="wkst")
            wk_insts = [
                nc.sync.dma_start(
                    wk_st[:, ft, :], wk_ext.ap()[ft * P : (ft + 1) * P, :]
                )
                for ft in range(FT)
            ]
            gate(wk_insts[:1], sp_insts)
            sp_prev = sp_insts + wk_insts

    # Wq + bq on the ACT queue, ungated (stream from t=0)
    wq_st = xstage.tile([P, FT, D], F32, tag="wstage", bufs=2, name="wqst")
    wq_insts = [
        nc.scalar.dma_start(wq_st[:, ft, :], wq_ext.ap()[ft * P : (ft + 1) * P, :])
        for ft in range(FT)
    ]
    bqf = consts.tile([P, FT], F32)
    nc.scalar.dma_start(bqf[:], bq_ext.ap().rearrange("(dt p) -> p dt", p=P))

    # persistent bf16 operands
    xT = persist.tile([P, FT, S], BF16, name="xT")       # [f(part), ftile, s]
    wT = [persist.tile([P, FT, D], BF16, name=f"wT{w}") for w in range(2)]
    wp = persist.tile([P, FT, D], BF16, name="wp")       # W' [f1(part), f1t, f2]
    tT = persist.tile([P, FT, S], BF16, name="tT")       # [f2(part), f2t, m]
    c2 = consts.tile([P, FT], F32)                       # bias per f2 partition
    bqb = consts.tile([P, FT], BF16)

    def tr_x(sg):
        # xT[ft][p, s] = x[s, ft*128+p] for this s-group; two f-tiles per
        # 2-bank PSUM tile, one merged [P,2,512] eviction each
        xts = xgroups[sg]
        for fp in range(2):
            ps = ps_tile(f"tr{sg}{fp}")
            for k in range(2):
                ft = 2 * fp + k
                for j in range(4):
                    nc.tensor.transpose(
                        ps[:, k, j * P : (j + 1) * P],
                        xts[:, j, ft * P : (ft + 1) * P],
                        ident[:],
                    )
            nc.vector.tensor_copy(
                xT[:, 2 * fp : 2 * fp + 2, sg * NCH : (sg + 1) * NCH], ps[:]
            )

    def tr_w(w, wst):
        # wT[w][p, dt, f] = W[f, dt*128+p]
        for dp in range(2):
            ps = ps_tile(f"wtr{w}{dp}")
            for k in range(2):
                dt = 2 * dp + k
                for ft in range(FT):
                    nc.tensor.transpose(
                        ps[:, k, ft * P : (ft + 1) * P],
                        wst[:, ft, dt * P : (dt + 1) * P],
                        ident[:],
                    )
            nc.scalar.activation(wT[w][:, 2 * dp : 2 * dp + 2, :], ps[:], Act.Identity)

    def emit_wprime():
        # W'[f1, f2] = sum_d Wq[f1, d] Wk[f2, d] = WqT^T @ WkT
        for fp in range(2):
            ps = ps_tile(f"wp{fp}")
            for k in range(2):
                f1c = 2 * fp + k
                for dt in range(FT):
                    nc.tensor.matmul(
                        ps[:, k, :],
                        wT[0][:, dt, f1c * P : (f1c + 1) * P],
                        wT[1][:, dt, :],
                        start=(dt == 0),
                        stop=(dt == FT - 1),
                    )
            nc.scalar.activation(wp[:, 2 * fp : 2 * fp + 2, :], ps[:], Act.Identity)
        # c2[f2] = sum_d Wk[f2, d] bq[d]; tiny N=1 matmuls, f32 PSUM accum
        nc.vector.tensor_copy(bqb[:], bqf[:])
        cps = ps_tile("c2ps")
        for f2c in range(FT):
            for dt in range(FT):
                nc.tensor.matmul(
                    cps[:, 0, f2c : f2c + 1],
                    wT[1][:, dt, f2c * P : (f2c + 1) * P],
                    bqb[:, dt : dt + 1],
                    start=(dt == 0),
                    stop=(dt == FT - 1),
                )
        nc.vector.tensor_copy(c2[:], cps[:, 0, :FT])

    def proj_t_pair(mg, fp, evict_act=True):
        # tT[f2, m] = sum_f1 W'[f1, f2] xT[f1, m] + c2[f2] for f2 chunk pair
        ps = ps_tile(f"pj{mg}{fp}")
        for k in range(2):
            f2c = 2 * fp + k
            for f1c in range(FT):
                nc.tensor.matmul(
                    ps[:, k, :],
                    wp[:, f1c, f2c * P : (f2c + 1) * P],
                    xT[:, f1c, mg * NCH : (mg + 1) * NCH],
                    start=(f1c == 0),
                    stop=(f1c == FT - 1),
                )
        for k in range(2):
            f2c = 2 * fp + k
            dst = tT[:, f2c, mg * NCH : (mg + 1) * NCH]
            bias = c2[:, f2c : f2c + 1]
            if evict_act:
                nc.scalar.activation(dst, ps[:, k, :], Act.Identity, bias=bias)
            else:
                nc.vector.tensor_scalar_add(dst, ps[:, k, :], bias)

    # --- pre-scores phase, ordered by expected DMA arrival:
    tr_x(0)
    tr_w(0, wq_st)
    tr_w(1, wk_st)
    emit_wprime()
    tr_x(1)
    proj_t_pair(0, 0)
    proj_t_pair(0, 1)
    tr_x(2)
    tr_x(3)

    # --- scores + softmax, one 128-row m-tile at a time; deferred tT
    # projections (groups 1-3) spread one f2-chunk-pair per 2 m-tiles.
    inv_sqrt_d = 1.0 / float(np.sqrt(np.float32(D)))
    for mt in range(ST):
        if mt < 12 and mt % 2 == 0:
            # tT group 1 during mt 0..3, group 2 during 4..7, group 3 during
            # 8..11; evict on DVE — ACT budget is exp-only in this phase
            proj_t_pair(mt // 4 + 1, (mt % 4) // 2, evict_act=False)
        pss = [ps_tile(f"ps{mt}_{i}") for i in range(2)]
        et = epool.tile([P, SC, NCH], F32)
        last_mt = mt == ST - 1
        asum = spool.tile([P, SC if last_mt else 2], F32, tag="asum")
        for ncn in range(SC):
            ps = pss[ncn // 2][:, ncn % 2, :]
            for f2c in range(FT):
                nc.tensor.matmul(
                    ps,
                    tT[:, f2c, mt * P : (mt + 1) * P],
                    xT[:, f2c, ncn * NCH : (ncn + 1) * NCH],
                    start=(f2c == 0),
                    stop=(f2c == FT - 1),
                )
            if last_mt:
                # finer exp chunks on the last m-tile: shorter drain chain
                nc.scalar.activation(
                    et[:, ncn, :],
                    ps,
                    Act.Exp,
                    scale=inv_sqrt_d,
                    accum_out=asum[:, ncn : ncn + 1],
                )
            elif ncn % 2 == 1:
                h = ncn // 2
                nc.scalar.activation(
                    et[:, 2 * h : 2 * h + 2, :],
                    pss[h][:],
                    Act.Exp,
                    scale=inv_sqrt_d,
                    accum_out=asum[:, h : h + 1],
                )
        rsum = spool.tile([P, 1], F32, tag="rsum")
        nc.vector.reduce_sum(rsum[:], asum[:], axis=mybir.AxisListType.X)
        rrec = spool.tile([P, 1], F32, tag="rrec")
        nc.vector.reciprocal(rrec[:], rsum[:])
        ot = opool.tile([P, SC, NCH], BF16 if OUT_BF16 else F32)
        if not last_mt:
            for h in range(2):
                nc.vector.tensor_scalar_mul(
                    ot[:, 2 * h : 2 * h + 2, :], et[:, 2 * h : 2 * h + 2, :], rrec[:]
                )
            # ONE merged output DMA per m-tile, SP/GpSimd alternating; ACT
            # stays exp-only so the epilogue keeps pace with the PE
            dma_eng = nc.sync if mt % 2 == 0 else nc.gpsimd
            dma_eng.dma_start(out_ext.ap()[mt * P : (mt + 1) * P, :], ot[:])
        else:
            # last m-tile: fine-grained drain — 512-wide normalize chunks
            # alternating DVE/ACT, output DMAs rotating SP/GpSimd
            for q in range(SC):
                sl = slice(q * NCH, (q + 1) * NCH)
                if q % 2 == 0:
                    nc.vector.tensor_scalar_mul(ot[:, q, :], et[:, q, :], rrec[:])
                else:
                    nc.scalar.activation(ot[:, q, :], et[:, q, :], Act.Identity, scale=rrec[:])
                dma_eng = nc.sync if q % 2 == 0 else nc.gpsimd
                dma_eng.dma_start(out_ext.ap()[mt * P : (mt + 1) * P, sl], ot[:, q, :])


_CACHE = {}


def build():
    if "nc" in _CACHE:
        return _CACHE["nc"]
    from contextlib import ExitStack

    nc = bacc.Bacc("TRN2", target_bir_lowering=False, debug=False, num_devices=B)
    x_ext = nc.dram_tensor("x", [S, F], F32, kind="ExternalInput")
    wq_ext = nc.dram_tensor("Wq", [F, D], F32, kind="ExternalInput")
    wk_ext = nc.dram_tensor("Wk", [F, D], F32, kind="ExternalInput")
    bq_ext = nc.dram_tensor("bq", [D], F32, kind="ExternalInput")
    out_ext = nc.dram_tensor(
        "out", [S, S], BF16 if OUT_BF16 else F32, kind="ExternalOutput"
    )

    with tile.TileContext(nc) as tc:
        with ExitStack() as ctx:
            _emit(nc, tc, ctx, x_ext, wq_ext, wk_ext, bq_ext, out_ext)

    nc.compile()
    _CACHE["nc"] = nc
    return nc


def make_in_maps(x, Wq, bq, Wk):
    x = np.ascontiguousarray(np.asarray(x, dtype=np.float32))
    Wq = np.ascontiguousarray(np.asarray(Wq, dtype=np.float32))
    Wk = np.ascontiguousarray(np.asarray(Wk, dtype=np.float32))
    bq = np.ascontiguousarray(np.asarray(bq, dtype=np.float32))
    return [{"x": x[i], "Wq": Wq, "Wk": Wk, "bq": bq} for i in range(B)]


def kernel(x, Wq, bq, Wk, bk=None, Wv=None, bv=None, **_unused):
    nc = build()
    in_maps = make_in_maps(x, Wq, bq, Wk)
    res = run_bass_kernel_spmd(nc, in_maps, core_ids=list(range(B)))
    return np.stack(
        [np.asarray(res.results[i]["out"], dtype=np.float32) for i in range(B)], axis=0
    )


# revision 9
# speedup vs baseline: 1.0468x; 1.0468x over previous
"""Trainium2 Bass kernel for nn_AttentionLayer: softmax(Q K^T / sqrt(d)).

Data-parallel over batch: 8 batch elements -> 8 NeuronCores, weights
replicated, no collectives.

Algebraic restructure (exact, softmax-invariant): with q = x Wq + bq and
k = x Wk + bk,
    q k^T = x (Wq Wk^T) x^T  +  1 (x Wk bq)^T  +  [terms constant along n]
and row-softmax drops any per-row constant, so
    alpha = softmax_n( (t x^T) / sqrt(d) ),   t = x W' + 1 c2^T,
    W' = Wq Wk^T  (512x512),  c2 = Wk bq.
This replaces one of the two [2048x512x512] projections with a single
[512x512x512] matmul (W') — ~6.6us less PE work per core — and removes
the bk load entirely.

Per core:
  xT    = transpose(x)            (PE f32 transposes, DVE evict->bf16)
  WqT/WkT = transpose(Wq/Wk)      (PE f32 transposes, ACT evict->bf16)
  W'    = WqT^T @ WkT             (TensorE bf16, ACT evict->bf16)
  c2    = WkT^T @ bq              (16 tiny N=1 matmuls, f32 PSUM accum)
  tT    = W'-chunks @ xT + c2     (TensorE bf16, bias evict ACT/DVE)
  S     = tT^T @ xT               (TensorE bf16, accumulate over f-tiles)
  E     = exp(S / sqrt(d)) with fused row-sum accumulate (ACT)
  out   = E / rowsum              (DVE per-partition scalar mul -> bf16)

Schedule notes (from NTFF traces): PE matmul throughput is at roofline
when dense (216ns per 512-wide bf16 MM). The input stream is the
startup constraint — ~6MB at the shared-HBM envelope takes ~25us — so
(a) the 2MB of weights loads FIRST, split over the SP/ACT/GpSimd
queues, because the serial W' -> tT chain depends on them, and (b) the
scores loop is split into half-tiles: each m-tile's n-chunks 0-1 touch
only x groups 0-1 and run while groups 2-3 are still streaming in; the
chunk 2-3 halves and the softmax epilogues follow once xg3 lands. This
keeps the PE dense from first data to last matmul. ACT stays exp-only
in the scores phase (output DMAs issue from SP and GpSimd-SWDGE,
normalization on DVE) so the epilogue keeps pace with the PE. The
end-of-kernel semaphore teardown (~12us for the framework-fixed 254
sems) and ~6us engine-init preamble are fixed costs. The DRAM output is
bf16 (halves the ~17MB/core output stream), upconverted to f32 on the
host; rel err vs the fp32 reference is ~4.5e-3.
"""

import os
import sys

sys.path.insert(0, "/opt/trn_rl_repo")

import numpy as np

import concourse.mybir as mybir
import concourse.tile as tile
from concourse import bacc
from concourse.bass_utils import run_bass_kernel_spmd
from concourse.masks import make_identity

B, S, F, D = 8, 2048, 512, 512
P = 128
ST = S // P   # 16 s-tiles
FT = F // P   # 4  f-tiles (contraction for projections / scores)
NCH = 512     # moving-operand / PSUM-bank chunk along the free axis
SC = S // NCH  # 4 chunks of the s axis
NSPLIT = 7    # m-tiles whose chunk-0/1 halves run ahead of xg3

F32 = mybir.dt.float32
BF16 = mybir.dt.bfloat16

WARMUP_MMS = int(os.environ.get("BASS_ATTN_WARMUP", "6"))
OUT_BF16 = os.environ.get("BASS_ATTN_OUT_BF16", "1") == "1"


def _emit(nc, tc, ctx, x_ext, wq_ext, wk_ext, bq_ext, out_ext):
    Act = mybir.ActivationFunctionType

    consts = ctx.enter_context(tc.tile_pool(name="consts", bufs=1))
    persist = ctx.enter_context(tc.tile_pool(name="persist", bufs=1))
    xstage = ctx.enter_context(tc.tile_pool(name="xstage", bufs=4))
    psum = ctx.enter_context(tc.tile_pool(name="psum", bufs=4, space="PSUM"))
    epool = ctx.enter_context(tc.tile_pool(name="epool", bufs=9))
    opool = ctx.enter_context(tc.tile_pool(name="opool", bufs=2))
    spool = ctx.enter_context(tc.tile_pool(name="spool", bufs=4))

    def ps_tile(name):
        # single unified PSUM tag: 4 bufs x [P, 2, 512] f32 = all 8 banks
        return psum.tile([P, 2, NCH], F32, tag="ps", bufs=4, name=name)

    ident = consts.tile([P, P], F32)
    make_identity(nc, ident[:])
    # --- PE warmup: garbage matmuls while input DMAs land (HAM -> K=8/8)
    if WARMUP_MMS:
        wps = ps_tile("warmps")
        for _ in range(WARMUP_MMS):
            nc.tensor.matmul(wps[:, 0, :P], ident[:], ident[:], start=True, stop=True)

    from concourse.tile import add_dep_helper

    def gate(first_insts, prev_insts):
        for fi in first_insts:
            for pi in prev_insts:
                add_dep_helper(fi.ins, pi.ins, reason="input DMA phase chain")

    # --- input streaming.  Per-queue concurrency bounds throughput (a
    # single transfer moves ~25GB/s, one queue ~100-250GB/s), and the
    # whole 6MB runs at the shared-HBM envelope either way, so order by
    # NEED: the 2MB of weights first (they gate the serial W' -> tT g0
    # chain), split over all three queues; then the x groups as 64-row
    # half-tiles, rows 0-63 on the SP chain and rows 64-127 on GpSimd.
    wq_st = xstage.tile([P, FT, D], F32, tag="wstage", bufs=2, name="wqst")
    wk_st = xstage.tile([P, FT, D], F32, tag="wstage", bufs=2, name="wkst")

    def wsub(eng, wst, w_ext, ft):
        return eng.dma_start(wst[:, ft, :], w_ext.ap()[ft * P : (ft + 1) * P, :])

    # ACT queue: Wq halves + bq (ungated, from t=0)
    wsub(nc.scalar, wq_st, wq_ext, 0)
    wsub(nc.scalar, wq_st, wq_ext, 1)
    bqf = consts.tile([P, FT], F32)
    nc.scalar.dma_start(bqf[:], bq_ext.ap().rearrange("(dt p) -> p dt", p=P))
    # SP queue: rest of Wq + half of Wk, then the x chain
    sp_prev = [
        wsub(nc.sync, wq_st, wq_ext, 2),
        wsub(nc.sync, wq_st, wq_ext, 3),
        wsub(nc.sync, wk_st, wk_ext, 0),
        wsub(nc.sync, wk_st, wk_ext, 1),
    ]
    # GpSimd (SWDGE) queue: rest of Wk, then the x chain
    gp_prev = [
        wsub(nc.gpsimd, wk_st, wk_ext, 2),
        wsub(nc.gpsimd, wk_st, wk_ext, 3),
    ]

    def load_x_group_half(t, sg, eng, half):
        insts = []
        lo, hi = (0, 64) if half == 0 else (64, P)
        for j in range(4):
            st = sg * 4 + j
            insts.append(
                eng.dma_start(
                    t[lo:hi, j, :], x_ext.ap()[st * P + lo : st * P + hi, :]
                )
            )
        return insts

    xgroups = {}
    for sg in range(SC):
        xgroups[sg] = xstage.tile([P, 4, F], F32, tag="xstage", bufs=4, name=f"xg{sg}")
        sp_insts = load_x_group_half(xgroups[sg], sg, nc.sync, 0)
        gp_insts = load_x_group_half(xgroups[sg], sg, nc.gpsimd, 1)
        gate(sp_insts[:1], sp_prev)
        gate(gp_insts[:1], gp_prev)
        sp_prev, gp_prev = sp_insts, gp_insts

    # persistent bf16 operands
    xT = persist.tile([P, FT, S], BF16, name="xT")       # [f(part), ftile, s]
    wT = [persist.tile([P, FT, D], BF16, name=f"wT{w}") for w in range(2)]
    wp = persist.tile([P, FT, D], BF16, name="wp")       # W' [f1(part), f1t, f2]
    tT = persist.tile([P, FT, S], BF16, name="tT")       # [f2(part), f2t, m]
    c2 = consts.tile([P, FT], F32)                       # bias per f2 partition
    bqb = consts.tile([P, FT], BF16)

    def tr_x(sg):
        # xT[ft][p, s] = x[s, ft*128+p] for this s-group; two f-tiles per
        # 2-bank PSUM tile, one merged [P,2,512] eviction each
        xts = xgroups[sg]
        for fp in range(2):
            ps = ps_tile(f"tr{sg}{fp}")
            for k in range(2):
                ft = 2 * fp + k
                for j in range(4):
                    nc.tensor.transpose(
                        ps[:, k, j * P : (j + 1) * P],
                        xts[:, j, ft * P : (ft + 1) * P],
                        ident[:],
                    )
            nc.vector.tensor_copy(
                xT[:, 2 * fp : 2 * fp + 2, sg * NCH : (sg + 1) * NCH], ps[:]
            )

    def tr_w(w, wst):
        # wT[w][p, dt, f] = W[f, dt*128+p]
        for dp in range(2):
            ps = ps_tile(f"wtr{w}{dp}")
            for k in range(2):
                dt = 2 * dp + k
                for ft in range(FT):
                    nc.tensor.transpose(
                        ps[:, k, ft * P : (ft + 1) * P],
                        wst[:, ft, dt * P : (dt + 1) * P],
                        ident[:],
                    )
            nc.scalar.activation(wT[w][:, 2 * dp : 2 * dp + 2, :], ps[:], Act.Identity)

    def emit_wprime():
        # W'[f1, f2] = sum_d Wq[f1, d] Wk[f2, d] = WqT^T @ WkT
        for fp in range(2):
            ps = ps_tile(f"wp{fp}")
            for k in range(2):
                f1c = 2 * fp + k
                for dt in range(FT):
                    nc.tensor.matmul(
                        ps[:, k, :],
                        wT[0][:, dt, f1c * P : (f1c + 1) * P],
                        wT[1][:, dt, :],
                        start=(dt == 0),
                        stop=(dt == FT - 1),
                    )
            nc.scalar.activation(wp[:, 2 * fp : 2 * fp + 2, :], ps[:], Act.Identity)
        # c2[f2] = sum_d Wk[f2, d] bq[d]; tiny N=1 matmuls, f32 PSUM accum
        nc.vector.tensor_copy(bqb[:], bqf[:])
        cps = ps_tile("c2ps")
        for f2c in range(FT):
            for dt in range(FT):
                nc.tensor.matmul(
                    cps[:, 0, f2c : f2c + 1],
                    wT[1][:, dt, f2c * P : (f2c + 1) * P],
                    bqb[:, dt : dt + 1],
                    start=(dt == 0),
                    stop=(dt == FT - 1),
                )
        nc.vector.tensor_copy(c2[:], cps[:, 0, :FT])

    def proj_t_pair(mg, fp, evict_act=True):
        # tT[f2, m] = sum_f1 W'[f1, f2] xT[f1, m] + c2[f2] for f2 chunk pair
        ps = ps_tile(f"pj{mg}{fp}")
        for k in range(2):
            f2c = 2 * fp + k
            for f1c in range(FT):
                nc.tensor.matmul(
                    ps[:, k, :],
                    wp[:, f1c, f2c * P : (f2c + 1) * P],
                    xT[:, f1c, mg * NCH : (mg + 1) * NCH],
                    start=(f1c == 0),
                    stop=(f1c == FT - 1),
                )
        for k in range(2):
            f2c = 2 * fp + k
            dst = tT[:, f2c, mg * NCH : (mg + 1) * NCH]
            bias = c2[:, f2c : f2c + 1]
            if evict_act:
                nc.scalar.activation(dst, ps[:, k, :], Act.Identity, bias=bias)
            else:
                nc.vector.tensor_scalar_add(dst, ps[:, k, :], bias)

    inv_sqrt_d = 1.0 / float(np.sqrt(np.float32(D)))
    ets = {}
    asums = {}

    def score_half(mt, h, et, asum):
        # chunks 2h, 2h+1 of m-tile mt: 8 MMs + one fused exp/accumulate
        ps = ps_tile(f"s{mt}_{h}")
        for k in range(2):
            ncn = 2 * h + k
            for f2c in range(FT):
                nc.tensor.matmul(
                    ps[:, k, :],
                    tT[:, f2c, mt * P : (mt + 1) * P],
                    xT[:, f2c, ncn * NCH : (ncn + 1) * NCH],
                    start=(f2c == 0),
                    stop=(f2c == FT - 1),
                )
        nc.scalar.activation(
            et[:, 2 * h : 2 * h + 2, :],
            ps[:],
            Act.Exp,
            scale=inv_sqrt_d,
            accum_out=asum[:, h : h + 1],
        )

    def epilogue(mt, et, asum):
        rsum = spool.tile([P, 1], F32, tag="rsum")
        nc.vector.reduce_sum(rsum[:], asum[:], axis=mybir.AxisListType.X)
        rrec = spool.tile([P, 1], F32, tag="rrec")
        nc.vector.reciprocal(rrec[:], rsum[:])
        ot = opool.tile([P, SC, NCH], BF16 if OUT_BF16 else F32)
        for h in range(2):
            nc.vector.tensor_scalar_mul(
                ot[:, 2 * h : 2 * h + 2, :], et[:, 2 * h : 2 * h + 2, :], rrec[:]
            )
        # ONE merged output DMA per m-tile, SP/GpSimd alternating; ACT
        # stays exp-only so the epilogue keeps pace with the PE
        dma_eng = nc.sync if mt % 2 == 0 else nc.gpsimd
        dma_eng.dma_start(out_ext.ap()[mt * P : (mt + 1) * P, :], ot[:])

    def new_et_asum(mt):
        ets[mt] = epool.tile([P, SC, NCH], F32, tag="et", bufs=9, name="et")
        asums[mt] = spool.tile([P, 2], F32, tag="asum", bufs=9, name="asum")

    # --- pre-scores: weight-derived chain first (weights land first)
    tr_w(0, wq_st)
    tr_w(1, wk_st)
    emit_wprime()
    tr_x(0)
    proj_t_pair(0, 0)
    proj_t_pair(0, 1)
    tr_x(1)

    # --- P1: chunk-0/1 halves of the first NSPLIT m-tiles (need only
    # x groups 0-1) run while x groups 2-3 are still streaming in;
    # deferred tT projections for m-groups 1-2 interleave (their moving
    # operand is x group 1 / 2 respectively).
    for mt in range(NSPLIT):
        new_et_asum(mt)
        score_half(mt, 0, ets[mt], asums[mt])
        if mt == 0:
            proj_t_pair(1, 0, evict_act=False)
        elif mt == 1:
            proj_t_pair(1, 1, evict_act=False)
        elif mt == 3:
            tr_x(2)
        elif mt == 4:
            proj_t_pair(2, 0, evict_act=False)
        elif mt == 5:
            proj_t_pair(2, 1, evict_act=False)
    tr_x(3)

    # --- P2: finish the split m-tiles (chunks 2-3 + epilogue), then the
    # remaining m-tiles in full; tT group 3 interleaves early in P2.
    for mt in range(NSPLIT):
        score_half(mt, 1, ets[mt], asums[mt])
        if mt == 0:
            proj_t_pair(3, 0, evict_act=False)
        elif mt == 1:
            proj_t_pair(3, 1, evict_act=False)
        epilogue(mt, ets[mt], asums[mt])
    for mt in range(NSPLIT, ST):
        last_mt = mt == ST - 1
        if not last_mt:
            new_et_asum(mt)
            score_half(mt, 0, ets[mt], asums[mt])
            score_half(mt, 1, ets[mt], asums[mt])
            epilogue(mt, ets[mt], asums[mt])
        else:
            # last m-tile: fine-grained drain — 512-wide exp/normalize
            # chunks, DMAs alternating GpSimd/SP (SP last)
            et = epool.tile([P, SC, NCH], F32, tag="et", bufs=9)
            asum = spool.tile([P, SC], F32, tag="asum", bufs=9)
            pss = [ps_tile(f"sl{i}") for i in range(2)]
            for ncn in range(SC):
                ps = pss[ncn // 2][:, ncn % 2, :]
                for f2c in range(FT):
                    nc.tensor.matmul(
                        ps,
                        tT[:, f2c, mt * P : (mt + 1) * P],
                        xT[:, f2c, ncn * NCH : (ncn + 1) * NCH],
                        start=(f2c == 0),
                        stop=(f2c == FT - 1),
                    )
                nc.scalar.activation(
                    et[:, ncn, :],
                    ps,
                    Act.Exp,
                    scale=inv_sqrt_d,
                    accum_out=asum[:, ncn : ncn + 1],
                )
            rsum = spool.tile([P, 1], F32, tag="rsum")
            nc.vector.reduce_sum(rsum[:], asum[:], axis=mybir.AxisListType.X)
            rrec = spool.tile([P, 1], F32, tag="rrec")
            nc.vector.reciprocal(rrec[:], rsum[:])
            ot = opool.tile([P, SC, NCH], BF16 if OUT_BF16 else F32)
            for q in range(SC):
                sl = slice(q * NCH, (q + 1) * NCH)
                if q % 2 == 0:
                    nc.vector.tensor_scalar_mul(ot[:, q, :], et[:, q, :], rrec[:])
                else:
                    nc.scalar.activation(ot[:, q, :], et[:, q, :], Act.Identity, scale=rrec[:])
                dma_eng = nc.gpsimd if q % 2 == 0 else nc.sync
                dma_eng.dma_start(out_ext.ap()[mt * P : (mt + 1) * P, sl], ot[:, q, :])


_CACHE = {}


def build():
    if "nc" in _CACHE:
        return _CACHE["nc"]
    from contextlib import ExitStack

    nc = bacc.Bacc("TRN2", target_bir_lowering=False, debug=False, num_devices=B)
    x_ext = nc.dram_tensor("x", [S, F], F32, kind="ExternalInput")
    wq_ext = nc.dram_tensor("Wq", [F, D], F32, kind="ExternalInput")
    wk_ext = nc.dram_tensor("Wk", [F, D], F32, kind="ExternalInput")
    bq_ext = nc.dram_tensor("bq", [D], F32, kind="ExternalInput")
    out_ext = nc.dram_tensor(
        "out", [S, S], BF16 if OUT_BF16 else F32, kind="ExternalOutput"
    )

    with tile.TileContext(nc) as tc:
        with ExitStack() as ctx:
            _emit(nc, tc, ctx, x_ext, wq_ext, wk_ext, bq_ext, out_ext)

    nc.compile()
    _CACHE["nc"] = nc
    return nc


def make_in_maps(x, Wq, bq, Wk):
    x = np.ascontiguousarray(np.asarray(x, dtype=np.float32))
    Wq = np.ascontiguousarray(np.asarray(Wq, dtype=np.float32))
    Wk = np.ascontiguousarray(np.asarray(Wk, dtype=np.float32))
    bq = np.ascontiguousarray(np.asarray(bq, dtype=np.float32))
    return [{"x": x[i], "Wq": Wq, "Wk": Wk, "bq": bq} for i in range(B)]


def kernel(x, Wq, bq, Wk, bk=None, Wv=None, bv=None, **_unused):
    nc = build()
    in_maps = make_in_maps(x, Wq, bq, Wk)
    res = run_bass_kernel_spmd(nc, in_maps, core_ids=list(range(B)))
    return np.stack(
        [np.asarray(res.results[i]["out"], dtype=np.float32) for i in range(B)], axis=0
    )
